# revision 15
# baseline (speedup 1.0000x reference)
"""Trainium2 Bass kernel for a 4-layer bidirectional Mamba (Caduceus) mixer.

Sharding: 8 cores = 2 directions x 4 channel-blocks of 128 (D_INNER=512).
Everything on-device runs in [channel, time] layout. Per layer:
  - residual update streams the previous AllReduce result with per-core
    direction masks (reverse cores read it through negative-stride APs)
  - rmsnorm weight is folded into the in_proj weights; the 1/rms row is
    applied after the matmul via a PE ones-broadcast
  - causal depthwise conv = 4 shifted scalar_tensor_tensor taps
  - x_proj partials are AllReduced within each direction group
  - the selective scan runs on the hardware tensor_tensor_scan (DVE),
    16 state columns x 8 time-slices, chained via per-state initials
  - the bidirectional merge + un-reversal is folded into a masked
    accumulation buffer consumed by the out_proj matmuls, followed by an
    8-core AllReduce that also performs the fwd+rev output sum
"""

import numpy as np

D_MODEL = 256
N_LAYER = 4
D_INNER = 512
D_STATE = 16
DT_RANK = 16
D_CONV = 4
VOCAB = 16
T = 4096
EPS = 1e-5
NCORES = 8
TB = 512          # time tile for matmuls / phase A
NT = T // TB      # 8
Q = 512           # time slice for the scan phase
NQ = T // Q       # 8
PAD = D_CONV - 1  # causal conv left padding

_CACHE = {}


def _build_program():
    import concourse.bacc as bacc
    import concourse.mybir as mybir
    from concourse import tile

    fp32 = mybir.dt.float32
    AL = mybir.AluOpType
    AF = mybir.ActivationFunctionType

    nc = bacc.Bacc("TRN2", target_bir_lowering=False, debug=False,
                   num_devices=NCORES)

    # ---- per-core inputs ----
    oh_d = nc.dram_tensor("oh", [VOCAB, T], fp32, kind="ExternalInput")
    embT_d = nc.dram_tensor("embT", [VOCAB, D_MODEL], fp32, kind="ExternalInput")
    mf_d = nc.dram_tensor("maskf", [128, 1], fp32, kind="ExternalInput")
    mr_d = nc.dram_tensor("maskr", [128, 1], fp32, kind="ExternalInput")
    w1T_d = nc.dram_tensor("w1T", [N_LAYER, 2, 128, 256], fp32, kind="ExternalInput")
    outwT_d = nc.dram_tensor("outwT", [N_LAYER, 128, 256], fp32, kind="ExternalInput")
    xpwT_d = nc.dram_tensor("xpwT", [N_LAYER, 128, 48], fp32, kind="ExternalInput")
    dtwT_d = nc.dram_tensor("dtwT", [N_LAYER, DT_RANK, 128], fp32, kind="ExternalInput")
    dtbW_d = nc.dram_tensor("dtbW", [N_LAYER, 1, 128], fp32, kind="ExternalInput")
    convW_d = nc.dram_tensor("convW", [N_LAYER, 128, D_CONV], fp32, kind="ExternalInput")
    cb_d = nc.dram_tensor("cb", [N_LAYER, 128, 1], fp32, kind="ExternalInput")
    A_d = nc.dram_tensor("Amat", [N_LAYER, 128, D_STATE], fp32, kind="ExternalInput")
    Dp_d = nc.dram_tensor("Dpv", [N_LAYER, 128, 1], fp32, kind="ExternalInput")
    nfw_d = nc.dram_tensor("nfw", [128, 2], fp32, kind="ExternalInput")
    selBC_d = nc.dram_tensor("selBC", [48, 2 * D_STATE * 128], fp32, kind="ExternalInput")
    out_d = nc.dram_tensor("out", [D_MODEL, T], fp32, kind="ExternalOutput")

    DIR_GROUPS = [[0, 1, 2, 3], [4, 5, 6, 7]]
    ALL_GROUP = [list(range(NCORES))]

    with tile.TileContext(nc, num_cores=NCORES) as tc:
        with (
            tc.tile_pool(name="const", bufs=1) as cpool,
            tc.tile_pool(name="persist", bufs=1) as ppool,
            tc.tile_pool(name="wpool", bufs=1) as wpool,
            tc.tile_pool(name="stream", bufs=2) as spool,
            tc.tile_pool(name="nloop", bufs=2) as npool,
            tc.tile_pool(name="dram", bufs=1, space="DRAM") as dpool,
        ):
            # ---- constants ----
            ones1 = cpool.tile([1, 128], fp32)       # broadcast lhsT
            invD = cpool.tile([128, 1], fp32)        # 1/256 column for mean
            ones_row = cpool.tile([1, TB], fp32)     # rhs for dtb matmul
            maskf = cpool.tile([128, 1], fp32)
            maskr = cpool.tile([128, 1], fp32)
            embT = cpool.tile([VOCAB, D_MODEL], fp32)
            nfw = cpool.tile([128, 2], fp32)
            epsc = cpool.tile([1, 1], fp32)
            nc.vector.memset(epsc[:], EPS)
            selBC = cpool.tile([48, 2 * D_STATE * 128], fp32)
            nc.sync.dma_start(selBC[:], selBC_d[:])
            nc.vector.memset(ones1[:], 1.0)
            nc.vector.memset(invD[:], 1.0)
            nc.vector.memset(ones_row[:], 1.0)
            nc.sync.dma_start(maskf[:], mf_d[:])
            nc.sync.dma_start(maskr[:], mr_d[:])
            nc.sync.dma_start(embT[:], embT_d[:])
            nc.sync.dma_start(nfw[:], nfw_d[:])

            # ---- persistent state ----
            R = [ppool.tile([128, T], fp32, name=f"resid{k}") for k in range(2)]
            nc.vector.memset(R[0][:], 0.0)
            nc.vector.memset(R[1][:], 0.0)
            dtT = ppool.tile([128, T], fp32)
            u_sil = ppool.tile([128, T], fp32)
            szT = ppool.tile([128, T], fp32)
            xdbT = ppool.tile([48, T], fp32)
            hlast = ppool.tile([128, D_STATE], fp32)

            # persistent DRAM bounce buffers
            h_cur = dpool.tile([2, 128, T], fp32)     # AR result / embedding
            h_bnc = dpool.tile([2, 128, T], fp32)     # out partial bounce
            xdb_in = dpool.tile([48, T], fp32)
            xdb_out = dpool.tile([48, T], fp32)

            # ---- embedding: h0_T = embT.T @ onehot -> h_cur ----
            with tc.tile_pool(name="ps_emb", bufs=2, space="PSUM") as ps_emb:
                for m in range(2):
                    for j in range(NT):
                        sl = slice(TB * j, TB * (j + 1))
                        ohl = spool.tile([VOCAB, TB], fp32, tag="ohld", bufs=1)
                        nc.sync.dma_start(ohl[:], oh_d[:, sl])
                        ep = ps_emb.tile([128, TB], fp32, tag="emb")
                        nc.tensor.matmul(ep[:], embT[:, 128 * m:128 * (m + 1)],
                                         ohl[:], start=True, stop=True)
                        es = spool.tile([128, TB], fp32, tag="cpy", bufs=1)
                        nc.scalar.copy(es[:], ep[:])
                        nc.sync.dma_start(h_cur[m, :, sl], es[:])

            for li in range(N_LAYER):
                # ---- per-layer weights ----
                w1T = [wpool.tile([128, 256], fp32, tag=f"w1T{k}",
                                  name=f"w1T{k}_{li}") for k in range(2)]
                nc.sync.dma_start(w1T[0][:], w1T_d[li, 0])
                nc.sync.dma_start(w1T[1][:], w1T_d[li, 1])
                outwT = wpool.tile([128, 256], fp32, tag="outwT")
                nc.sync.dma_start(outwT[:], outwT_d[li])
                xpwT = wpool.tile([128, 48], fp32, tag="xpwT")
                nc.sync.dma_start(xpwT[:], xpwT_d[li])
                dtwT = wpool.tile([DT_RANK, 128], fp32, tag="dtwT")
                nc.sync.dma_start(dtwT[:], dtwT_d[li])
                dtbW = wpool.tile([1, 128], fp32, tag="dtbW")
                nc.sync.dma_start(dtbW[:], dtbW_d[li])
                convW = wpool.tile([128, D_CONV], fp32, tag="convW")
                nc.sync.dma_start(convW[:], convW_d[li])
                cbt = wpool.tile([128, 1], fp32, tag="cbt")
                nc.sync.dma_start(cbt[:], cb_d[li])
                Amat = wpool.tile([128, D_STATE], fp32, tag="Amat")
                nc.sync.dma_start(Amat[:], A_d[li])
                Dpt = wpool.tile([128, 1], fp32, tag="Dpt")
                nc.sync.dma_start(Dpt[:], Dp_d[li])

                # conv input scratch, reused as the masked out_proj rhs later
                u_sc = ppool.tile([128, T + PAD], fp32, tag="big_scratch",
                                  name=f"u_sc{li}")
                nc.vector.memset(u_sc[:, 0:PAD], 0.0)

                # ---- residual update: R += h*mf + rev(h)*mr ----
                for k in range(2):
                    for j in range(NT):
                        sl = slice(TB * j, TB * (j + 1))
                        rsl = slice(TB * (NT - 1 - j), TB * (NT - j))
                        ha = spool.tile([128, TB], fp32, tag="hldA")
                        hb = spool.tile([128, TB], fp32, tag="hldB")
                        nc.sync.dma_start(ha[:], h_cur[k, :, sl])
                        nc.sync.dma_start(hb[:], h_cur[k, :, rsl])
                        nc.vector.scalar_tensor_tensor(
                            R[k][:, sl], ha[:], maskf[:], R[k][:, sl],
                            AL.mult, AL.add)
                        nc.vector.scalar_tensor_tensor(
                            R[k][:, sl], hb[:][:, ::-1], maskr[:], R[k][:, sl],
                            AL.mult, AL.add)

                # ---- rms stats: 1/sqrt(mean(R^2)+eps) ----
                rrow = spool.tile([1, T], fp32, tag="rrow", bufs=1)
                with tc.tile_pool(name="psA", bufs=1, space="PSUM") as psA:
                    msA = psA.tile([1, T], fp32, tag="msA")
                    for j in range(NT):
                        sl = slice(TB * j, TB * (j + 1))
                        for k in range(2):
                            sq = spool.tile([128, TB], fp32, tag="sq")
                            nc.scalar.square(sq[:], R[k][:, sl])
                            nc.tensor.matmul(msA[0:1, sl], invD[:], sq[:],
                                             start=(k == 0), stop=(k == 1))
                    nc.scalar.activation(rrow[:], msA[:], AF.Ln,
                                         bias=epsc[:], scale=1.0 / D_MODEL)
                nc.scalar.activation(rrow[:], rrow[:], AF.Exp,
                                     bias=0.0, scale=-0.5)

                # ---- in_proj + row scaling ----
                with tc.tile_pool(name="psB", bufs=2, space="PSUM") as psB:
                    for j in range(NT):
                        sl = slice(TB * j, TB * (j + 1))
                        rrep = psB.tile([128, TB], fp32, tag="rrep", bufs=1)
                        nc.tensor.matmul(rrep[:], ones1[:], rrow[0:1, sl],
                                         start=True, stop=True)
                        rrs = spool.tile([128, TB], fp32, tag="rrs")
                        nc.scalar.copy(rrs[:], rrep[:])
                        xzu = psB.tile([128, TB], fp32, tag="xzu")
                        xzz = psB.tile([128, TB], fp32, tag="xzz")
                        for k in range(2):
                            nc.tensor.matmul(xzu[:], w1T[k][:, 0:128],
                                             R[k][:, sl],
                                             start=(k == 0), stop=(k == 1))
                        for k in range(2):
                            nc.tensor.matmul(xzz[:], w1T[k][:, 128:256],
                                             R[k][:, sl],
                                             start=(k == 0), stop=(k == 1))
                        nc.vector.tensor_tensor(
                            u_sc[:, PAD + TB * j:PAD + TB * (j + 1)],
                            xzu[:], rrs[:], AL.mult)
                        zs = spool.tile([128, TB], fp32, tag="zs", bufs=1)
                        nc.vector.tensor_tensor(zs[:], xzz[:], rrs[:], AL.mult)
                        nc.scalar.activation(szT[:, sl], zs[:], AF.Silu)

                # ---- conv + silu + x_proj partial ----
                with tc.tile_pool(name="psC", bufs=2, space="PSUM") as psC:
                    for j in range(NT):
                        sl = slice(TB * j, TB * (j + 1))
                        acc = [None, None]
                        for kk in range(D_CONV):
                            src = u_sc[:, kk + TB * j: kk + TB * j + TB]
                            dst = spool.tile([128, TB], fp32, bufs=1,
                                             tag=f"cacc{kk % 2}")
                            if kk == 0:
                                nc.vector.tensor_scalar_mul(
                                    dst[:], src, convW[:, 0:1])
                            else:
                                nc.vector.scalar_tensor_tensor(
                                    dst[:], src, convW[:, kk:kk + 1],
                                    acc[(kk - 1) % 2][:], AL.mult, AL.add)
                            acc[kk % 2] = dst
                        nc.scalar.activation(u_sil[:, sl],
                                             acc[(D_CONV - 1) % 2][:],
                                             AF.Silu, bias=cbt[:], scale=1.0)
                        xp = psC.tile([48, TB], fp32, tag="xp")
                        nc.tensor.matmul(xp[:], xpwT[:], u_sil[:, sl],
                                         start=True, stop=True)
                        xps = spool.tile([48, TB], fp32, tag="xps")
                        nc.scalar.copy(xps[:], xp[:])
                        nc.sync.dma_start(xdb_in[:, sl], xps[:])

                # ---- x_proj AllReduce within direction group ----
                nc.gpsimd.collective_compute(
                    "AllReduce", AL.add, replica_groups=DIR_GROUPS,
                    ins=[xdb_in[:].opt()], outs=[xdb_out[:].opt()])
                nc.sync.dma_start(xdbT[:], xdb_out[:])

                # ---- dt_proj + softplus ----
                with tc.tile_pool(name="psD", bufs=2, space="PSUM") as psD:
                    for j in range(NT):
                        sl = slice(TB * j, TB * (j + 1))
                        dtp = psD.tile([128, TB], fp32, tag="dtp")
                        nc.tensor.matmul(dtp[:], dtwT[:], xdbT[0:DT_RANK, sl],
                                         start=True, stop=False)
                        nc.tensor.matmul(dtp[:], dtbW[:], ones_row[:],
                                         start=False, stop=True)
                        et = spool.tile([128, TB], fp32, tag="et", bufs=1)
                        nc.scalar.activation(et[:], dtp[:], AF.Exp)
                        nc.vector.tensor_scalar_add(et[:], et[:], 1.0)
                        nc.scalar.activation(dtT[:, sl], et[:], AF.Ln)

                # ---- scan phase: 16 states x NQ time slices ----
                ygm = u_sc  # big scratch: cols [0:T] hold the masked rhs
                nc.vector.memset(ygm[:, 0:T], 0.0)
                with tc.tile_pool(name="psE", bufs=2, space="PSUM") as psE:
                    for q in range(NQ):
                        qsl = slice(Q * q, Q * (q + 1))
                        dtu = npool.tile([128, Q], fp32, tag="dtu", bufs=1)
                        nc.vector.tensor_tensor(dtu[:], dtT[:, qsl],
                                                u_sil[:, qsl], AL.mult)
                        ya = None
                        for n in range(D_STATE):
                            dA = npool.tile([128, Q], fp32, tag="dA")
                            nc.scalar.activation(dA[:], dtT[:, qsl], AF.Exp,
                                                 bias=0.0,
                                                 scale=Amat[:, n:n + 1])
                            Bq = psE.tile([128, Q], fp32, tag="Bq")
                            nc.tensor.matmul(Bq[:],
                                             selBC[:, 128 * n:128 * (n + 1)],
                                             xdbT[:, qsl],
                                             start=True, stop=True)
                            dBu = npool.tile([128, Q], fp32, tag="dBu")
                            nc.vector.tensor_tensor(dBu[:], dtu[:], Bq[:],
                                                    AL.mult)
                            ht = npool.tile([128, Q], fp32, tag="ht")
                            init = 0.0 if q == 0 else hlast[:, n:n + 1]
                            nc.vector.tensor_tensor_scan(
                                ht[:], dA[:], dBu[:], init, AL.mult, AL.add)
                            if q < NQ - 1:
                                nc.vector.tensor_copy(hlast[:, n:n + 1],
                                                      ht[:, Q - 1:Q])
                            Cq = psE.tile([128, Q], fp32, tag="Cq")
                            nc.tensor.matmul(
                                Cq[:],
                                selBC[:, 128 * (D_STATE + n):
                                      128 * (D_STATE + n + 1)],
                                xdbT[:, qsl], start=True, stop=True)
                            yn = npool.tile([128, Q], fp32, tag=f"yp{n % 2}",
                                            bufs=1)
                            if n == 0:
                                nc.vector.tensor_tensor(yn[:], ht[:], Cq[:],
                                                        AL.mult)
                            else:
                                yt = psE.tile([128, Q], fp32, tag="yt")
                                nc.vector.tensor_tensor(yt[:], ht[:], Cq[:],
                                                        AL.mult)
                                nc.vector.tensor_tensor(yn[:], yt[:], ya[:],
                                                        AL.add)
                            ya = yn
                        # skip + gate
                        yg = npool.tile([128, Q], fp32, tag="dBu")
                        nc.vector.scalar_tensor_tensor(
                            yg[:], u_sil[:, qsl], Dpt[:], ya[:],
                            AL.mult, AL.add)
                        yg2 = npool.tile([128, Q], fp32, tag="dA")
                        nc.vector.tensor_tensor(yg2[:], yg[:], szT[:, qsl],
                                                AL.mult)
                        # masked write into ygm (fwd) / reversed ygm (rev)
                        rqsl = slice(T - Q * (q + 1), T - Q * q)
                        nc.vector.scalar_tensor_tensor(
                            ygm[:, qsl], yg2[:], maskf[:],
                            ygm[:, qsl], AL.mult, AL.add)
                        nc.vector.scalar_tensor_tensor(
                            ygm[:, rqsl], yg2[:][:, ::-1], maskr[:],
                            ygm[:, rqsl], AL.mult, AL.add)

                # ---- out_proj partial -> AllReduce over all 8 cores ----
                with tc.tile_pool(name="psF", bufs=2, space="PSUM") as psF:
                    for m in range(2):
                        for j in range(NT):
                            sl = slice(TB * j, TB * (j + 1))
                            op = psF.tile([128, TB], fp32, tag="op")
                            nc.tensor.matmul(op[:],
                                             outwT[:, 128 * m:128 * (m + 1)],
                                             ygm[:, sl],
                                             start=True, stop=True)
                            ops = spool.tile([128, TB], fp32, tag="cpy", bufs=1)
                            nc.scalar.copy(ops[:], op[:])
                            nc.sync.dma_start(h_bnc[m, :, sl], ops[:])
                nc.gpsimd.collective_compute(
                    "AllReduce", AL.add, replica_groups=ALL_GROUP,
                    ins=[h_bnc[:].opt()], outs=[h_cur[:].opt()])

            # ---- final: resid += h, rmsnorm with norm_f_w ----
            for k in range(2):
                for j in range(NT):
                    sl = slice(TB * j, TB * (j + 1))
                    rsl = slice(TB * (NT - 1 - j), TB * (NT - j))
                    ha = spool.tile([128, TB], fp32, tag="hldA")
                    hb = spool.tile([128, TB], fp32, tag="hldB")
                    nc.sync.dma_start(ha[:], h_cur[k, :, sl])
                    nc.sync.dma_start(hb[:], h_cur[k, :, rsl])
                    nc.vector.scalar_tensor_tensor(
                        R[k][:, sl], ha[:], maskf[:], R[k][:, sl],
                        AL.mult, AL.add)
                    nc.vector.scalar_tensor_tensor(
                        R[k][:, sl], hb[:][:, ::-1], maskr[:], R[k][:, sl],
                        AL.mult, AL.add)
            rrow = spool.tile([1, T], fp32, tag="rrow", bufs=1)
            with tc.tile_pool(name="psG0", bufs=1, space="PSUM") as psG0:
                msA = psG0.tile([1, T], fp32, tag="msAf")
                for j in range(NT):
                    sl = slice(TB * j, TB * (j + 1))
                    for k in range(2):
                        sq = spool.tile([128, TB], fp32, tag="sq")
                        nc.scalar.square(sq[:], R[k][:, sl])
                        nc.tensor.matmul(msA[0:1, sl], invD[:], sq[:],
                                         start=(k == 0), stop=(k == 1))
                nc.scalar.activation(rrow[:], msA[:], AF.Ln,
                                     bias=epsc[:], scale=1.0 / D_MODEL)
            nc.scalar.activation(rrow[:], rrow[:], AF.Exp,
                                 bias=0.0, scale=-0.5)
            with tc.tile_pool(name="psG", bufs=2, space="PSUM") as psG:
                for j in range(NT):
                    sl = slice(TB * j, TB * (j + 1))
                    rrep = psG.tile([128, TB], fp32, tag="rrepf")
                    nc.tensor.matmul(rrep[:], ones1[:], rrow[0:1, sl],
                                     start=True, stop=True)
                    rrs = spool.tile([128, TB], fp32, tag="rrs")
                    nc.scalar.copy(rrs[:], rrep[:])
                    for k in range(2):
                        ot = spool.tile([128, TB], fp32, tag="ot")
                        nc.vector.scalar_tensor_tensor(
                            ot[:], R[k][:, sl], nfw[:, k:k + 1], rrs[:],
                            AL.mult, AL.mult)
                        nc.sync.dma_start(out_d[128 * k:128 * (k + 1), sl],
                                          ot[:])

    nc.compile()
    return nc


def _prep_inputs(inputs):
    ids = np.asarray(inputs["input_ids"])[0]
    oh = np.zeros((VOCAB, T), np.float32)
    oh[ids, np.arange(T)] = 1.0
    embT = np.ascontiguousarray(np.asarray(inputs["embed_w"], np.float32))
    W1p = np.einsum('led,ld->led', np.asarray(inputs["in_proj_w"], np.float32),
                    np.asarray(inputs["norm_w"], np.float32))
    A = -np.exp(np.asarray(inputs["A_log"], np.float32))
    out_w = np.asarray(inputs["out_proj_w"], np.float32)
    xpw = np.asarray(inputs["x_proj_w"], np.float32)
    dtw = np.asarray(inputs["dt_proj_w"], np.float32)
    dtb = np.asarray(inputs["dt_proj_b"], np.float32)
    cw = np.asarray(inputs["conv_w"], np.float32)
    cb = np.asarray(inputs["conv_b"], np.float32)
    Dp = np.asarray(inputs["Dp"], np.float32)
    nfw = np.asarray(inputs["norm_f_w"], np.float32)

    in_maps = []
    for core in range(NCORES):
        g, b = core // 4, core % 4
        db = slice(128 * b, 128 * (b + 1))
        w1T = np.empty((N_LAYER, 2, 128, 256), np.float32)
        outwT = np.empty((N_LAYER, 128, 256), np.float32)
        xpwT = np.empty((N_LAYER, 128, 48), np.float32)
        dtwT = np.empty((N_LAYER, DT_RANK, 128), np.float32)
        dtbW = np.empty((N_LAYER, 1, 128), np.float32)
        for li in range(N_LAYER):
            rows = np.concatenate(
                [W1p[li, db, :],
                 W1p[li, D_INNER + 128 * b:D_INNER + 128 * (b + 1), :]])
            for k in range(2):
                w1T[li, k] = rows[:, 128 * k:128 * (k + 1)].T
            outwT[li] = out_w[li][:, db].T
            xpwT[li] = xpw[li, g][:, db].T
            dtwT[li] = dtw[li, g, db, :].T
            dtbW[li, 0] = dtb[li, g, db]
        selBC = np.zeros((48, 2 * D_STATE * 128), np.float32)
        for n in range(D_STATE):
            selBC[16 + n, 128 * n:128 * (n + 1)] = 1.0
            selBC[32 + n, 128 * (D_STATE + n):128 * (D_STATE + n + 1)] = 1.0
        mofs = 1.0 if g == 0 else 0.0
        in_maps.append({
            "oh": oh, "embT": embT,
            "maskf": np.full((128, 1), mofs, np.float32),
            "maskr": np.full((128, 1), 1.0 - mofs, np.float32),
            "w1T": w1T, "outwT": outwT, "xpwT": xpwT,
            "dtwT": dtwT, "dtbW": dtbW,
            "convW": np.ascontiguousarray(cw[:, g, db, :]),
            "cb": np.ascontiguousarray(cb[:, g, db, None]),
            "Amat": np.ascontiguousarray(A[:, g, db, :]),
            "Dpv": np.ascontiguousarray(Dp[:, g, db, None]),
            "nfw": np.ascontiguousarray(nfw.reshape(2, 128).T),
            "selBC": selBC,
        })
    return in_maps


def _get_program():
    if "nc" not in _CACHE:
        _CACHE["nc"] = _build_program()
    return _CACHE["nc"]


def kernel(**inputs):
    from concourse.bass_utils import run_bass_kernel_spmd
    nc = _get_program()
    in_maps = _prep_inputs(inputs)
    res = run_bass_kernel_spmd(nc, in_maps, list(range(NCORES)))
    out_T = res.results[0]["out"]
    return np.ascontiguousarray(out_T.T[None]).astype(np.float32)


# revision 16
# speedup vs baseline: 32.1212x; 32.1212x over previous
"""Trainium2 Bass kernel for a 4-layer bidirectional Mamba (Caduceus) mixer.

Sharding: 8 cores = 2 directions x 4 channel-blocks of 128 (D_INNER=512).
Everything on-device runs in [channel, time] layout. Per layer:
  - residual update streams the previous AllReduce result with per-core
    direction masks (reverse cores read it through negative-stride APs)
  - rmsnorm weight is folded into the in_proj weights; the 1/rms row is
    applied after the matmul via a PE ones-broadcast
  - causal depthwise conv = 4 shifted scalar_tensor_tensor taps
  - x_proj partials are AllReduced within each direction group
  - the selective scan runs on the hardware tensor_tensor_scan (DVE),
    16 state columns x 8 time-slices, chained via per-state initials
  - the bidirectional merge + un-reversal is folded into a masked
    accumulation buffer consumed by the out_proj matmuls, followed by an
    8-core AllReduce that also performs the fwd+rev output sum
"""

import numpy as np

D_MODEL = 256
N_LAYER = 4
D_INNER = 512
D_STATE = 16
DT_RANK = 16
D_CONV = 4
VOCAB = 16
T = 4096
EPS = 1e-5
NCORES = 8
TB = 512          # time tile for matmuls / phase A
NT = T // TB      # 8
Q = 512           # time slice for the scan phase
NQ = T // Q       # 8
PAD = D_CONV - 1  # causal conv left padding

_CACHE = {}


def _build_program():
    import concourse.bacc as bacc
    import concourse.mybir as mybir
    from concourse import tile

    fp32 = mybir.dt.float32
    AL = mybir.AluOpType
    AF = mybir.ActivationFunctionType

    nc = bacc.Bacc("TRN2", target_bir_lowering=False, debug=False,
                   num_devices=NCORES)

    # ---- per-core inputs ----
    oh_d = nc.dram_tensor("oh", [VOCAB, T], fp32, kind="ExternalInput")
    embT_d = nc.dram_tensor("embT", [VOCAB, D_MODEL], fp32, kind="ExternalInput")
    mf_d = nc.dram_tensor("maskf", [128, 1], fp32, kind="ExternalInput")
    mr_d = nc.dram_tensor("maskr", [128, 1], fp32, kind="ExternalInput")
    w1T_d = nc.dram_tensor("w1T", [N_LAYER, 2, 128, 256], fp32, kind="ExternalInput")
    outwT_d = nc.dram_tensor("outwT", [N_LAYER, 128, 256], fp32, kind="ExternalInput")
    xpwT_d = nc.dram_tensor("xpwT", [N_LAYER, 128, 48], fp32, kind="ExternalInput")
    dtwT_d = nc.dram_tensor("dtwT", [N_LAYER, DT_RANK, 128], fp32, kind="ExternalInput")
    dtbW_d = nc.dram_tensor("dtbW", [N_LAYER, 1, 128], fp32, kind="ExternalInput")
    convW_d = nc.dram_tensor("convW", [N_LAYER, 128, D_CONV], fp32, kind="ExternalInput")
    cb_d = nc.dram_tensor("cb", [N_LAYER, 128, 1], fp32, kind="ExternalInput")
    A_d = nc.dram_tensor("Amat", [N_LAYER, 128, D_STATE], fp32, kind="ExternalInput")
    Dp_d = nc.dram_tensor("Dpv", [N_LAYER, 128, 1], fp32, kind="ExternalInput")
    nfw_d = nc.dram_tensor("nfw", [128, 2], fp32, kind="ExternalInput")
    selBC_d = nc.dram_tensor("selBC", [48, 2 * D_STATE * 128], fp32, kind="ExternalInput")
    out_d = nc.dram_tensor("out", [D_MODEL, T], fp32, kind="ExternalOutput")

    DIR_GROUPS = [[0, 1, 2, 3], [4, 5, 6, 7]]
    ALL_GROUP = [list(range(NCORES))]

    with tile.TileContext(nc, num_cores=NCORES) as tc:
        with (
            tc.tile_pool(name="const", bufs=1) as cpool,
            tc.tile_pool(name="persist", bufs=1) as ppool,
            tc.tile_pool(name="wpool", bufs=1) as wpool,
            tc.tile_pool(name="stream", bufs=2) as spool,
            tc.tile_pool(name="nloop", bufs=2) as npool,
            tc.tile_pool(name="dram", bufs=1, space="DRAM") as dpool,
        ):
            # ---- constants ----
            ones1 = cpool.tile([1, 128], fp32)       # broadcast lhsT
            invD = cpool.tile([128, 1], fp32)        # 1/256 column for mean
            ones_row = cpool.tile([1, TB], fp32)     # rhs for dtb matmul
            maskf = cpool.tile([128, 1], fp32)
            maskr = cpool.tile([128, 1], fp32)
            embT = cpool.tile([VOCAB, D_MODEL], fp32)
            nfw = cpool.tile([128, 2], fp32)
            epsc = cpool.tile([1, 1], fp32)
            nc.vector.memset(epsc[:], EPS)
            selBC = cpool.tile([48, 2 * D_STATE * 128], fp32)
            nc.sync.dma_start(selBC[:], selBC_d[:])
            nc.vector.memset(ones1[:], 1.0)
            nc.vector.memset(invD[:], 1.0)
            nc.vector.memset(ones_row[:], 1.0)
            nc.sync.dma_start(maskf[:], mf_d[:])
            nc.sync.dma_start(maskr[:], mr_d[:])
            nc.sync.dma_start(embT[:], embT_d[:])
            nc.sync.dma_start(nfw[:], nfw_d[:])

            # ---- persistent state ----
            R = [ppool.tile([128, T], fp32, name=f"resid{k}") for k in range(2)]
            nc.vector.memset(R[0][:], 0.0)
            nc.vector.memset(R[1][:], 0.0)
            dtT = ppool.tile([128, T], fp32)
            u_sil = ppool.tile([128, T], fp32)
            szT = ppool.tile([128, T], fp32)
            xdbT = ppool.tile([48, T], fp32)
            hlast = ppool.tile([128, D_STATE], fp32)

            # persistent DRAM bounce buffers
            h_cur = dpool.tile([2, 128, T], fp32)     # AR result / embedding
            h_bnc = dpool.tile([2, 128, T], fp32)     # out partial bounce
            xdb_in = dpool.tile([48, T], fp32)
            xdb_out = dpool.tile([48, T], fp32)

            # ---- embedding: h0_T = embT.T @ onehot -> h_cur ----
            with tc.tile_pool(name="ps_emb", bufs=2, space="PSUM") as ps_emb:
                for m in range(2):
                    for j in range(NT):
                        sl = slice(TB * j, TB * (j + 1))
                        ohl = spool.tile([VOCAB, TB], fp32, tag="ohld", bufs=1)
                        nc.sync.dma_start(ohl[:], oh_d[:, sl])
                        ep = ps_emb.tile([128, TB], fp32, tag="emb")
                        nc.tensor.matmul(ep[:], embT[:, 128 * m:128 * (m + 1)],
                                         ohl[:], start=True, stop=True)
                        es = spool.tile([128, TB], fp32, tag="cpy", bufs=1)
                        nc.scalar.copy(es[:], ep[:])
                        nc.sync.dma_start(h_cur[m, :, sl], es[:])

            for li in range(N_LAYER):
                # ---- per-layer weights ----
                w1T = [wpool.tile([128, 256], fp32, tag=f"w1T{k}",
                                  name=f"w1T{k}_{li}") for k in range(2)]
                nc.sync.dma_start(w1T[0][:], w1T_d[li, 0])
                nc.sync.dma_start(w1T[1][:], w1T_d[li, 1])
                outwT = wpool.tile([128, 256], fp32, tag="outwT")
                nc.sync.dma_start(outwT[:], outwT_d[li])
                xpwT = wpool.tile([128, 48], fp32, tag="xpwT")
                nc.sync.dma_start(xpwT[:], xpwT_d[li])
                dtwT = wpool.tile([DT_RANK, 128], fp32, tag="dtwT")
                nc.sync.dma_start(dtwT[:], dtwT_d[li])
                dtbW = wpool.tile([1, 128], fp32, tag="dtbW")
                nc.sync.dma_start(dtbW[:], dtbW_d[li])
                convW = wpool.tile([128, D_CONV], fp32, tag="convW")
                nc.sync.dma_start(convW[:], convW_d[li])
                cbt = wpool.tile([128, 1], fp32, tag="cbt")
                nc.sync.dma_start(cbt[:], cb_d[li])
                Amat = wpool.tile([128, D_STATE], fp32, tag="Amat")
                nc.sync.dma_start(Amat[:], A_d[li])
                Dpt = wpool.tile([128, 1], fp32, tag="Dpt")
                nc.sync.dma_start(Dpt[:], Dp_d[li])

                # conv input scratch, reused as the masked out_proj rhs later
                u_sc = ppool.tile([128, T + PAD], fp32, tag="big_scratch",
                                  name=f"u_sc{li}")
                nc.vector.memset(u_sc[:, 0:PAD], 0.0)

                # ---- residual update: R += h*mf + rev(h)*mr ----
                for k in range(2):
                    for j in range(NT):
                        sl = slice(TB * j, TB * (j + 1))
                        rsl = slice(TB * (NT - 1 - j), TB * (NT - j))
                        ha = spool.tile([128, TB], fp32, tag="hldA")
                        hb = spool.tile([128, TB], fp32, tag="hldB")
                        nc.sync.dma_start(ha[:], h_cur[k, :, sl])
                        nc.sync.dma_start(hb[:], h_cur[k, :, rsl])
                        nc.vector.scalar_tensor_tensor(
                            R[k][:, sl], ha[:], maskf[:], R[k][:, sl],
                            AL.mult, AL.add)
                        nc.vector.scalar_tensor_tensor(
                            R[k][:, sl], hb[:][:, ::-1], maskr[:], R[k][:, sl],
                            AL.mult, AL.add)

                # ---- rms stats: 1/sqrt(mean(R^2)+eps) ----
                rrow = spool.tile([1, T], fp32, tag="rrow", bufs=1)
                with tc.tile_pool(name="psA", bufs=1, space="PSUM") as psA:
                    msA = psA.tile([1, T], fp32, tag="msA")
                    for j in range(NT):
                        sl = slice(TB * j, TB * (j + 1))
                        for k in range(2):
                            sq = spool.tile([128, TB], fp32, tag="sq")
                            nc.scalar.square(sq[:], R[k][:, sl])
                            nc.tensor.matmul(msA[0:1, sl], invD[:], sq[:],
                                             start=(k == 0), stop=(k == 1))
                    nc.scalar.activation(rrow[:], msA[:], AF.Ln,
                                         bias=epsc[:], scale=1.0 / D_MODEL)
                nc.scalar.activation(rrow[:], rrow[:], AF.Exp,
                                     bias=0.0, scale=-0.5)

                # ---- in_proj + row scaling ----
                with tc.tile_pool(name="psB", bufs=2, space="PSUM") as psB:
                    for j in range(NT):
                        sl = slice(TB * j, TB * (j + 1))
                        rrep = psB.tile([128, TB], fp32, tag="rrep", bufs=1)
                        nc.tensor.matmul(rrep[:], ones1[:], rrow[0:1, sl],
                                         start=True, stop=True)
                        rrs = spool.tile([128, TB], fp32, tag="rrs")
                        nc.scalar.copy(rrs[:], rrep[:])
                        xzu = psB.tile([128, TB], fp32, tag="xzu")
                        xzz = psB.tile([128, TB], fp32, tag="xzz")
                        for k in range(2):
                            nc.tensor.matmul(xzu[:], w1T[k][:, 0:128],
                                             R[k][:, sl],
                                             start=(k == 0), stop=(k == 1))
                        for k in range(2):
                            nc.tensor.matmul(xzz[:], w1T[k][:, 128:256],
                                             R[k][:, sl],
                                             start=(k == 0), stop=(k == 1))
                        nc.vector.tensor_tensor(
                            u_sc[:, PAD + TB * j:PAD + TB * (j + 1)],
                            xzu[:], rrs[:], AL.mult)
                        zs = spool.tile([128, TB], fp32, tag="zs", bufs=1)
                        nc.vector.tensor_tensor(zs[:], xzz[:], rrs[:], AL.mult)
                        nc.scalar.activation(szT[:, sl], zs[:], AF.Silu)

                # ---- conv + silu + x_proj partial ----
                with tc.tile_pool(name="psC", bufs=2, space="PSUM") as psC:
                    for j in range(NT):
                        sl = slice(TB * j, TB * (j + 1))
                        acc = [None, None]
                        for kk in range(D_CONV):
                            src = u_sc[:, kk + TB * j: kk + TB * j + TB]
                            dst = spool.tile([128, TB], fp32, bufs=1,
                                             tag=f"cacc{kk % 2}")
                            if kk == 0:
                                nc.vector.tensor_scalar_mul(
                                    dst[:], src, convW[:, 0:1])
                            else:
                                nc.vector.scalar_tensor_tensor(
                                    dst[:], src, convW[:, kk:kk + 1],
                                    acc[(kk - 1) % 2][:], AL.mult, AL.add)
                            acc[kk % 2] = dst
                        nc.scalar.activation(u_sil[:, sl],
                                             acc[(D_CONV - 1) % 2][:],
                                             AF.Silu, bias=cbt[:], scale=1.0)
                        xp = psC.tile([48, TB], fp32, tag="xp")
                        nc.tensor.matmul(xp[:], xpwT[:], u_sil[:, sl],
                                         start=True, stop=True)
                        xps = spool.tile([48, TB], fp32, tag="xps")
                        nc.scalar.copy(xps[:], xp[:])
                        nc.sync.dma_start(xdb_in[:, sl], xps[:])

                # ---- x_proj AllReduce within direction group ----
                nc.gpsimd.collective_compute(
                    "AllReduce", AL.add, replica_groups=DIR_GROUPS,
                    ins=[xdb_in[:].opt()], outs=[xdb_out[:].opt()])
                nc.sync.dma_start(xdbT[:], xdb_out[:])

                # ---- dt_proj + softplus ----
                with tc.tile_pool(name="psD", bufs=2, space="PSUM") as psD:
                    for j in range(NT):
                        sl = slice(TB * j, TB * (j + 1))
                        dtp = psD.tile([128, TB], fp32, tag="dtp")
                        nc.tensor.matmul(dtp[:], dtwT[:], xdbT[0:DT_RANK, sl],
                                         start=True, stop=False)
                        nc.tensor.matmul(dtp[:], dtbW[:], ones_row[:],
                                         start=False, stop=True)
                        et = spool.tile([128, TB], fp32, tag="et", bufs=1)
                        nc.scalar.activation(et[:], dtp[:], AF.Exp)
                        nc.vector.tensor_scalar_add(et[:], et[:], 1.0)
                        nc.scalar.activation(dtT[:, sl], et[:], AF.Ln)

                # ---- scan phase: 16 states x NQ time slices ----
                ygm = u_sc  # big scratch: cols [0:T] hold the masked rhs
                nc.vector.memset(ygm[:, 0:T], 0.0)
                with tc.tile_pool(name="psE", bufs=2, space="PSUM") as psE:
                    for q in range(NQ):
                        qsl = slice(Q * q, Q * (q + 1))
                        dtu = npool.tile([128, Q], fp32, tag="dtu", bufs=1)
                        nc.vector.tensor_tensor(dtu[:], dtT[:, qsl],
                                                u_sil[:, qsl], AL.mult)
                        ya = None
                        for n in range(D_STATE):
                            dA = npool.tile([128, Q], fp32, tag="dA")
                            nc.scalar.activation(dA[:], dtT[:, qsl], AF.Exp,
                                                 bias=0.0,
                                                 scale=Amat[:, n:n + 1])
                            Bq = psE.tile([128, Q], fp32, tag="Bq")
                            nc.tensor.matmul(Bq[:],
                                             selBC[:, 128 * n:128 * (n + 1)],
                                             xdbT[:, qsl],
                                             start=True, stop=True)
                            dBu = npool.tile([128, Q], fp32, tag="dBu")
                            nc.vector.tensor_tensor(dBu[:], dtu[:], Bq[:],
                                                    AL.mult)
                            ht = npool.tile([128, Q], fp32, tag="ht")
                            init = 0.0 if q == 0 else hlast[:, n:n + 1]
                            nc.vector.tensor_tensor_scan(
                                ht[:], dA[:], dBu[:], init, AL.mult, AL.add)
                            if q < NQ - 1:
                                nc.vector.tensor_copy(hlast[:, n:n + 1],
                                                      ht[:, Q - 1:Q])
                            Cq = psE.tile([128, Q], fp32, tag="Cq")
                            nc.tensor.matmul(
                                Cq[:],
                                selBC[:, 128 * (D_STATE + n):
                                      128 * (D_STATE + n + 1)],
                                xdbT[:, qsl], start=True, stop=True)
                            yn = npool.tile([128, Q], fp32, tag=f"yp{n % 2}",
                                            bufs=1)
                            if n == 0:
                                nc.vector.tensor_tensor(yn[:], ht[:], Cq[:],
                                                        AL.mult)
                            else:
                                yt = psE.tile([128, Q], fp32, tag="yt")
                                nc.vector.tensor_tensor(yt[:], ht[:], Cq[:],
                                                        AL.mult)
                                nc.vector.tensor_tensor(yn[:], yt[:], ya[:],
                                                        AL.add)
                            ya = yn
                        # skip + gate
                        yg = npool.tile([128, Q], fp32, tag="dBu")
                        nc.vector.scalar_tensor_tensor(
                            yg[:], u_sil[:, qsl], Dpt[:], ya[:],
                            AL.mult, AL.add)
                        yg2 = npool.tile([128, Q], fp32, tag="dA")
                        nc.vector.tensor_tensor(yg2[:], yg[:], szT[:, qsl],
                                                AL.mult)
                        # masked write into ygm (fwd) / reversed ygm (rev)
                        rqsl = slice(T - Q * (q + 1), T - Q * q)
                        nc.vector.scalar_tensor_tensor(
                            ygm[:, qsl], yg2[:], maskf[:],
                            ygm[:, qsl], AL.mult, AL.add)
                        nc.vector.scalar_tensor_tensor(
                            ygm[:, rqsl], yg2[:][:, ::-1], maskr[:],
                            ygm[:, rqsl], AL.mult, AL.add)

                # ---- out_proj partial -> AllReduce over all 8 cores ----
                with tc.tile_pool(name="psF", bufs=2, space="PSUM") as psF:
                    for m in range(2):
                        for j in range(NT):
                            sl = slice(TB * j, TB * (j + 1))
                            op = psF.tile([128, TB], fp32, tag="op")
                            nc.tensor.matmul(op[:],
                                             outwT[:, 128 * m:128 * (m + 1)],
                                             ygm[:, sl],
                                             start=True, stop=True)
                            ops = spool.tile([128, TB], fp32, tag="cpy", bufs=1)
                            nc.scalar.copy(ops[:], op[:])
                            nc.sync.dma_start(h_bnc[m, :, sl], ops[:])
                nc.gpsimd.collective_compute(
                    "AllReduce", AL.add, replica_groups=ALL_GROUP,
                    ins=[h_bnc[:].opt()], outs=[h_cur[:].opt()])

            # ---- final: resid += h, rmsnorm with norm_f_w ----
            for k in range(2):
                for j in range(NT):
                    sl = slice(TB * j, TB * (j + 1))
                    rsl = slice(TB * (NT - 1 - j), TB * (NT - j))
                    ha = spool.tile([128, TB], fp32, tag="hldA")
                    hb = spool.tile([128, TB], fp32, tag="hldB")
                    nc.sync.dma_start(ha[:], h_cur[k, :, sl])
                    nc.sync.dma_start(hb[:], h_cur[k, :, rsl])
                    nc.vector.scalar_tensor_tensor(
                        R[k][:, sl], ha[:], maskf[:], R[k][:, sl],
                        AL.mult, AL.add)
                    nc.vector.scalar_tensor_tensor(
                        R[k][:, sl], hb[:][:, ::-1], maskr[:], R[k][:, sl],
                        AL.mult, AL.add)
            rrow = spool.tile([1, T], fp32, tag="rrow", bufs=1)
            with tc.tile_pool(name="psG0", bufs=1, space="PSUM") as psG0:
                msA = psG0.tile([1, T], fp32, tag="msAf")
                for j in range(NT):
                    sl = slice(TB * j, TB * (j + 1))
                    for k in range(2):
                        sq = spool.tile([128, TB], fp32, tag="sq")
                        nc.scalar.square(sq[:], R[k][:, sl])
                        nc.tensor.matmul(msA[0:1, sl], invD[:], sq[:],
                                         start=(k == 0), stop=(k == 1))
                nc.scalar.activation(rrow[:], msA[:], AF.Ln,
                                     bias=epsc[:], scale=1.0 / D_MODEL)
            nc.scalar.activation(rrow[:], rrow[:], AF.Exp,
                                 bias=0.0, scale=-0.5)
            with tc.tile_pool(name="psG", bufs=2, space="PSUM") as psG:
                for j in range(NT):
                    sl = slice(TB * j, TB * (j + 1))
                    rrep = psG.tile([128, TB], fp32, tag="rrepf")
                    nc.tensor.matmul(rrep[:], ones1[:], rrow[0:1, sl],
                                     start=True, stop=True)
                    rrs = spool.tile([128, TB], fp32, tag="rrs")
                    nc.scalar.copy(rrs[:], rrep[:])
                    for k in range(2):
                        ot = spool.tile([128, TB], fp32, tag="ot")
                        nc.vector.scalar_tensor_tensor(
                            ot[:], R[k][:, sl], nfw[:, k:k + 1], rrs[:],
                            AL.mult, AL.mult)
                        nc.sync.dma_start(out_d[128 * k:128 * (k + 1), sl],
                                          ot[:])

    nc.compile()
    return nc


def _prep_inputs(inputs):
    ids = np.asarray(inputs["input_ids"])[0]
    oh = np.zeros((VOCAB, T), np.float32)
    oh[ids, np.arange(T)] = 1.0
    embT = np.ascontiguousarray(np.asarray(inputs["embed_w"], np.float32))
    W1p = np.einsum('led,ld->led', np.asarray(inputs["in_proj_w"], np.float32),
                    np.asarray(inputs["norm_w"], np.float32))
    A = -np.exp(np.asarray(inputs["A_log"], np.float32))
    out_w = np.asarray(inputs["out_proj_w"], np.float32)
    xpw = np.asarray(inputs["x_proj_w"], np.float32)
    dtw = np.asarray(inputs["dt_proj_w"], np.float32)
    dtb = np.asarray(inputs["dt_proj_b"], np.float32)
    cw = np.asarray(inputs["conv_w"], np.float32)
    cb = np.asarray(inputs["conv_b"], np.float32)
    Dp = np.asarray(inputs["Dp"], np.float32)
    nfw = np.asarray(inputs["norm_f_w"], np.float32)

    in_maps = []
    for core in range(NCORES):
        g, b = core // 4, core % 4
        db = slice(128 * b, 128 * (b + 1))
        w1T = np.empty((N_LAYER, 2, 128, 256), np.float32)
        outwT = np.empty((N_LAYER, 128, 256), np.float32)
        xpwT = np.empty((N_LAYER, 128, 48), np.float32)
        dtwT = np.empty((N_LAYER, DT_RANK, 128), np.float32)
        dtbW = np.empty((N_LAYER, 1, 128), np.float32)
        for li in range(N_LAYER):
            rows = np.concatenate(
                [W1p[li, db, :],
                 W1p[li, D_INNER + 128 * b:D_INNER + 128 * (b + 1), :]])
            for k in range(2):
                w1T[li, k] = rows[:, 128 * k:128 * (k + 1)].T
            outwT[li] = out_w[li][:, db].T
            xpwT[li] = xpw[li, g][:, db].T
            dtwT[li] = dtw[li, g, db, :].T
            dtbW[li, 0] = dtb[li, g, db]
        selBC = np.zeros((48, 2 * D_STATE * 128), np.float32)
        for n in range(D_STATE):
            selBC[16 + n, 128 * n:128 * (n + 1)] = 1.0
            selBC[32 + n, 128 * (D_STATE + n):128 * (D_STATE + n + 1)] = 1.0
        mofs = 1.0 if g == 0 else 0.0
        in_maps.append({
            "oh": oh, "embT": embT,
            "maskf": np.full((128, 1), mofs, np.float32),
            "maskr": np.full((128, 1), 1.0 - mofs, np.float32),
            "w1T": w1T, "outwT": outwT, "xpwT": xpwT,
            "dtwT": dtwT, "dtbW": dtbW,
            "convW": np.ascontiguousarray(cw[:, g, db, :]),
            "cb": np.ascontiguousarray(cb[:, g, db, None]),
            "Amat": np.ascontiguousarray(A[:, g, db, :]),
            "Dpv": np.ascontiguousarray(Dp[:, g, db, None]),
            "nfw": np.ascontiguousarray(nfw.reshape(2, 128).T),
            "selBC": selBC,
        })
    return in_maps


def _get_program():
    if "nc" not in _CACHE:
        _CACHE["nc"] = _build_program()
    return _CACHE["nc"]


def _get_runner():
    """Persistent jitted SPMD callable (avoids per-call retrace)."""
    if "runner" in _CACHE:
        return _CACHE["runner"]
    import jax
    import concourse.mybir as mybir
    from concourse import bass2jax
    from concourse.bass2jax import _bass_exec_p, install_neuronx_cc_hook
    from jax.sharding import Mesh, PartitionSpec
    from jax.experimental.shard_map import shard_map

    nc = _get_program()
    install_neuronx_cc_hook()
    partition_name = (nc.partition_id_tensor.name
                      if nc.partition_id_tensor else None)
    in_names, out_names, out_avals, zero_outs = [], [], [], []
    for alloc in nc.m.functions[0].allocations:
        if not isinstance(alloc, mybir.MemoryLocationSet):
            continue
        name = alloc.memorylocations[0].name
        if alloc.kind == "ExternalInput":
            if name != partition_name:
                in_names.append(name)
        elif alloc.kind == "ExternalOutput":
            shape = tuple(alloc.tensor_shape)
            dtype = mybir.dt.np(alloc.dtype)
            out_names.append(name)
            out_avals.append(jax.core.ShapedArray(shape, dtype))
            zero_outs.append(np.zeros(shape, dtype))
    n_params = len(in_names)
    all_names = list(in_names) + list(out_names)
    if partition_name is not None:
        all_names.append(partition_name)

    def _body(*args):
        operands = list(args)
        if partition_name is not None:
            operands.append(bass2jax.partition_id_tensor())
        outs = _bass_exec_p.bind(
            *operands, out_avals=tuple(out_avals), in_names=tuple(all_names),
            out_names=tuple(out_names), lowering_input_output_aliases=(),
            sim_require_finite=True, sim_require_nnan=True, nc=nc)
        return tuple(outs)

    devices = jax.devices()[:NCORES]
    mesh = Mesh(np.asarray(devices), ("core",))
    nin = n_params + len(out_names)
    sharded = jax.jit(
        shard_map(_body, mesh=mesh, in_specs=(PartitionSpec("core"),) * nin,
                  out_specs=(PartitionSpec("core"),) * len(out_names),
                  check_rep=False),
        keep_unused=True)

    def run(in_maps):
        concat_in = [np.concatenate([np.asarray(m[n]) for m in in_maps],
                                    axis=0) for n in in_names]
        concat_zero = [np.zeros((NCORES * z.shape[0], *z.shape[1:]), z.dtype)
                       for z in zero_outs]
        out_arrs = sharded(*concat_in, *concat_zero)
        return {
            name: np.asarray(out_arrs[i]).reshape(
                NCORES, *out_avals[i].shape)[0]
            for i, name in enumerate(out_names)
        }

    _CACHE["runner"] = (run, sharded, in_names, zero_outs)
    return _CACHE["runner"]


def kernel(**inputs):
    run = _get_runner()[0]
    in_maps = _prep_inputs(inputs)
    res = run(in_maps)
    out_T = res["out"]
    return np.ascontiguousarray(out_T.T[None]).astype(np.float32)
